# revision 39
# baseline (speedup 1.0000x reference)
"""Trainium2 Bass kernel for GNN message passing + GRU + MLP head.

Strategy:
  - Sort edges by destination on host; nodes split into 8 ranges of 1280,
    one per NeuronCore -> no collectives.
  - Edges packed into 128-edge tiles grouped by 32-node destination
    windows; windows processed in quads (4 PSUM col-strips) so scatter
    matmuls overlap via tile_position col groups.
  - All matmuls in fp16 (fp32 matmul runs 2-pass LOW_HIGH on TRN2; fp16
    is single-pass with enough mantissa for ~1e-3 end-to-end error).
  - Per tile: lin1 matmul (K=33 incl bias row) -> relu (ACT/DVE split,
    f32 PSUM -> fp16 SBUF) -> one-hot scatter matmul accumulated into
    PSUM-resident agg (has_written cleared by K=1 dummy matmuls).
  - agg transposed on-chip (PE transpose) to feed GRU matmuls; GRU gates
    + MLP head in [H, nodes] layout; outputs un-transposed on host.
"""

import numpy as np
import ml_dtypes

import concourse.bass as bass
import concourse.tile as tile
from concourse import bacc, mybir
from concourse.bass_utils import run_bass_kernel_spmd
from concourse.masks import make_identity

N_NODES = 10000
N_EDGES = 320000
D_IN = 32
H = 256
N_CORES = 8
WIN = 32
NODES_PER_CORE = 1280
N_WIN = NODES_PER_CORE // WIN    # 40 windows/core
NBLK = N_WIN // 4                # 10 window quads
P = 128
F32 = mybir.dt.float32
F16 = mybir.dt.float16
NP16 = np.float16
BF16 = mybir.dt.bfloat16
NPBF = ml_dtypes.bfloat16

_CACHE = {}


def _preprocess(x, edge_index):
    """Sort/partition edges; build per-core packed fp16 inputs."""
    row = np.asarray(edge_index[0], dtype=np.int64)
    col = np.asarray(edge_index[1], dtype=np.int64)
    order = np.argsort(col, kind="stable")
    col_s = col[order]
    row_s = row[order]

    n_win_glob = N_CORES * N_WIN
    bounds = np.searchsorted(col_s, np.arange(0, n_win_glob + 1) * WIN)
    cnt = bounds[1:] - bounds[:-1]
    T_w = max(1, int(np.max((cnt + P - 1) // P)))

    n_tiles = N_WIN * T_w
    e_slot = n_tiles * P
    cores = []
    for c in range(N_CORES):
        src = np.full(e_slot, -1, dtype=np.int64)
        offs = np.full(e_slot, 64.0, dtype=np.float32)
        # tile order: j (quad) -> t (slot) -> q (window in quad)
        idx = 0
        for j in range(NBLK):
            for t in range(T_w):
                for q in range(4):
                    g = 4 * j + q
                    w = c * N_WIN + g
                    lo, hi = bounds[w], bounds[w + 1]
                    s0 = lo + t * P
                    s1 = min(s0 + P, hi)
                    k = s1 - s0
                    if k > 0:
                        b = idx * P
                        src[b:b + k] = row_s[s0:s1]
                        offs[b:b + k] = (col_s[s0:s1] - w * WIN).astype(np.float32)
                    idx += 1
        valid = src >= 0
        xg = np.zeros((e_slot, 33), dtype=np.float32)
        xg[valid, :D_IN] = x[src[valid]]
        xg[:, D_IN] = 1.0  # bias row
        t3 = xg.reshape(n_tiles, P, 33).transpose(0, 2, 1)  # [T, 33, 128]
        pack = np.zeros((n_tiles // 2, 128, 128), dtype=np.float32)
        pack[:, 0:33, :] = t3[0::2]
        pack[:, 33:66, :] = t3[1::2]
        xg_pack = pack.transpose(1, 0, 2).reshape(128, (n_tiles // 2) * 128)
        offs_arr = offs.reshape(n_tiles, P).T  # [128, T]
        cores.append({
            "xg": np.ascontiguousarray(xg_pack.astype(NP16)),
            "offs": np.ascontiguousarray(offs_arr.astype(NP16)),
        })
    return T_w, cores


def _build_graph(T_w):
    n_tiles = N_WIN * T_w
    xg_cols = (n_tiles // 2) * 128
    NPAD = NODES_PER_CORE

    nc = bacc.Bacc()
    d_xg = nc.declare_dram_parameter("xg", [128, xg_cols], F16, isOutput=False)
    d_offs = nc.declare_dram_parameter("offs", [128, n_tiles], F16, isOutput=False)
    d_iota = nc.declare_dram_parameter("iota", [128, 256], F16, isOutput=False)
    d_l1w = nc.declare_dram_parameter("l1w", [66, 512], F16, isOutput=False)
    d_ht = nc.declare_dram_parameter("ht", [256, NPAD], F16, isOutput=False)
    d_xt = nc.declare_dram_parameter("xt", [32, NPAD], F16, isOutput=False)
    d_wrz = nc.declare_dram_parameter("wrz", [512, 512], F16, isOutput=False)
    d_win = nc.declare_dram_parameter("win", [256, 256], F16, isOutput=False)
    d_whn = nc.declare_dram_parameter("whn", [256, 256], F16, isOutput=False)
    d_hgw = nc.declare_dram_parameter("hgw", [256, 256], F16, isOutput=False)
    d_gawx = nc.declare_dram_parameter("gawx", [32, 1], F16, isOutput=False)
    d_gawg = nc.declare_dram_parameter("gawg", [256, 1], F16, isOutput=False)
    d_brz = nc.declare_dram_parameter("brz", [512, 1], F32, isOutput=False)
    d_bin = nc.declare_dram_parameter("bin", [256, 1], F32, isOutput=False)
    d_bhn = nc.declare_dram_parameter("bhn", [256, 1], F32, isOutput=False)
    d_hgb = nc.declare_dram_parameter("hgb", [256, 1], F32, isOutput=False)
    d_gab = nc.declare_dram_parameter("gab", [1, 1], F32, isOutput=False)
    d_hout = nc.declare_dram_parameter("h_out", [256, NPAD], F16, isOutput=True)
    d_aout = nc.declare_dram_parameter("a_out", [1, NPAD], F32, isOutput=True)

    AF = mybir.ActivationFunctionType
    OP = mybir.AluOpType

    with tile.TileContext(nc) as tc:
        with (
            tc.tile_pool(name="const", bufs=1) as cpool,
            tc.tile_pool(name="ps8", bufs=1, space="PSUM") as ps8,
            tc.tile_pool(name="xgc", bufs=3) as xg_pool,
            tc.tile_pool(name="ohb", bufs=4) as oh_pool,
            tc.tile_pool(name="msgs", bufs=6) as msg_pool,
            tc.tile_pool(name="gsb", bufs=14) as gsb,
        ):
            # ---- first xg tiles first: minimize PE start latency ----
            xg_first = xg_pool.tile([128, 8 * 128], F16, tag="xgf", bufs=1)
            for pc in range(4):
                nc.sync.dma_start(xg_first[:, 256 * pc:256 * pc + 256],
                                  d_xg[:, 256 * pc:256 * pc + 256])
            # ---- constants / inputs (sync queue: edge data first) ----
            iota_s = cpool.tile([128, 256], F16)
            nc.sync.dma_start(iota_s[:], d_iota[:])
            l1w_s = cpool.tile([66, 512], F16)  # block-diag for tile pairs
            nc.sync.dma_start(l1w_s[:], d_l1w[:])
            offs_s = cpool.tile([128, n_tiles], F16)
            nc.sync.dma_start(offs_s[:], d_offs[:])
            zc = cpool.tile([1, 128], F16)
            nc.vector.memset(zc[:], 0.0)
            zr = cpool.tile([1, 512], F16)
            nc.vector.memset(zr[:], 0.0)

            # ---- GRU weights / node inputs (gpsimd queue) ----
            ht_s = cpool.tile([128, 2 * NPAD], F16)
            nc.gpsimd.dma_start(
                ht_s[:].rearrange("p (k n) -> p k n", k=2),
                d_ht[:].rearrange("(k p) n -> p k n", p=128))
            xt_s = cpool.tile([32, NPAD], F16)
            nc.gpsimd.dma_start(xt_s[:], d_xt[:])
            wrz_s = cpool.tile([128, 2048], F16)
            nc.gpsimd.dma_start(
                wrz_s[:].rearrange("p (k m) -> p k m", k=4),
                d_wrz[:].rearrange("(k p) m -> p k m", p=128))
            win_s = cpool.tile([128, 512], F16)
            nc.gpsimd.dma_start(
                win_s[:].rearrange("p (k m) -> p k m", k=2),
                d_win[:].rearrange("(k p) m -> p k m", p=128))
            whn_s = cpool.tile([128, 512], F16)
            nc.gpsimd.dma_start(
                whn_s[:].rearrange("p (k m) -> p k m", k=2),
                d_whn[:].rearrange("(k p) m -> p k m", p=128))
            hgw_s = cpool.tile([128, 512], F16)
            nc.gpsimd.dma_start(
                hgw_s[:].rearrange("p (k m) -> p k m", k=2),
                d_hgw[:].rearrange("(k p) m -> p k m", p=128))
            gawx_s = cpool.tile([32, 1], F16)
            nc.gpsimd.dma_start(gawx_s[:], d_gawx[:])
            gawg_s = cpool.tile([128, 2], F16)
            nc.gpsimd.dma_start(
                gawg_s[:].rearrange("p (k o) -> p k o", k=2),
                d_gawg[:].rearrange("(k p) o -> p k o", p=128))
            brz_s = cpool.tile([128, 4], F32)
            nc.gpsimd.dma_start(
                brz_s[:].rearrange("p (k o) -> p k o", k=4),
                d_brz[:].rearrange("(k p) o -> p k o", p=128))
            bin_s = cpool.tile([128, 2], F32)
            nc.gpsimd.dma_start(
                bin_s[:].rearrange("p (k o) -> p k o", k=2),
                d_bin[:].rearrange("(k p) o -> p k o", p=128))
            bhn_s = cpool.tile([128, 2], F32)
            nc.gpsimd.dma_start(
                bhn_s[:].rearrange("p (k o) -> p k o", k=2),
                d_bhn[:].rearrange("(k p) o -> p k o", p=128))
            hgb_s = cpool.tile([128, 2], F32)
            nc.gpsimd.dma_start(
                hgb_s[:].rearrange("p (k o) -> p k o", k=2),
                d_hgb[:].rearrange("(k p) o -> p k o", p=128))
            gab_s = cpool.tile([1, 1], F32)
            nc.gpsimd.dma_start(gab_s[:], d_gab[:])

            # ---- persistent SBUF state ----
            aggT = [cpool.tile([128, NPAD], F16, name=f"aggT{i}") for i in range(2)]
            hts = [ht_s[:, :NPAD], ht_s[:, NPAD:]]
            hnew_s = cpool.tile([128, 2 * NPAD], F16)
            hnews = [hnew_s[:, :NPAD], hnew_s[:, NPAD:]]
            a_s = cpool.tile([1, NPAD], F32)

            # ---- agg PSUM banks (slots recycle into GRU psum) ----
            agg_b = [ps8.tile([128, 512], F32, tag="agg", bufs=5, name=f"agg_b{k}")
                     for k in range(5)]
            for k in range(5):  # clear has_written bits
                nc.tensor.matmul(agg_b[k][:], lhsT=zc[:], rhs=zr[:],
                                 start=True, stop=False, skip_group_check=True)

            kstack = [aggT[0][:], aggT[1][:], hts[0], hts[1]]
            NCHUNK = [(0, 512), (512, 1024), (1024, NPAD)]

            def emit_gru_chunk(ci):
                n0, n1 = NCHUNK[ci]
                ncn = n1 - n0
                rz_ps = []
                for m in range(4):
                    ps = ps8.tile([128, 512], F32, tag="agg", bufs=5, name=f"rz{ci}{m}")
                    for k in (2, 3, 0, 1):  # h-side first
                        nc.tensor.matmul(
                            ps[:, :ncn], lhsT=wrz_s[:, 512 * k + 128 * m:512 * k + 128 * m + 128],
                            rhs=kstack[k][:, n0:n1], start=(k == 2), stop=(k == 1))
                    rz_ps.append(ps)
                hn_ps = []
                for m in range(2):
                    ps = ps8.tile([128, 512], F32, tag="agg", bufs=5, name=f"hn{ci}{m}")
                    for k in range(2):
                        nc.tensor.matmul(
                            ps[:, :ncn], lhsT=whn_s[:, 256 * k + 128 * m:256 * k + 128 * m + 128],
                            rhs=hts[k][:, n0:n1], start=(k == 0), stop=(k == 1))
                    hn_ps.append(ps)
                in_ps = []
                for m in range(2):
                    ps = ps8.tile([128, 512], F32, tag="agg", bufs=5, name=f"in{ci}{m}")
                    for k in range(2):
                        nc.tensor.matmul(
                            ps[:, :ncn], lhsT=win_s[:, 256 * k + 128 * m:256 * k + 128 * m + 128],
                            rhs=aggT[k][:, n0:n1], start=(k == 0), stop=(k == 1))
                    in_ps.append(ps)

                n_sb = []
                z_sb = []
                for m in range(2):
                    r_m = gsb.tile([128, 512], F16, tag="g")
                    nc.scalar.activation(r_m[:, :ncn], rz_ps[m][:, :ncn],
                                         AF.Sigmoid, bias=brz_s[:, m:m + 1])
                    z_m = gsb.tile([128, 512], F16, tag="g")
                    nc.scalar.activation(z_m[:, :ncn], rz_ps[2 + m][:, :ncn],
                                         AF.Sigmoid, bias=brz_s[:, 2 + m:3 + m])
                    z_sb.append(z_m)
                    t1 = gsb.tile([128, 512], F16, tag="g")
                    nc.vector.scalar_tensor_tensor(
                        out=t1[:, :ncn], in0=hn_ps[m][:, :ncn],
                        scalar=bhn_s[:, m:m + 1], in1=r_m[:, :ncn],
                        op0=OP.add, op1=OP.mult)
                    t2 = gsb.tile([128, 512], F16, tag="g")
                    nc.vector.tensor_tensor(out=t2[:, :ncn], in0=t1[:, :ncn],
                                            in1=in_ps[m][:, :ncn], op=OP.add)
                    n_m = gsb.tile([128, 512], F16, tag="g")
                    nc.scalar.activation(n_m[:, :ncn], t2[:, :ncn],
                                         AF.Tanh, bias=bin_s[:, m:m + 1])
                    n_sb.append(n_m)
                for m in range(2):
                    d_m = gsb.tile([128, 512], F16, tag="g")
                    nc.vector.tensor_tensor(out=d_m[:, :ncn], in0=hts[m][:, n0:n1],
                                            in1=n_sb[m][:, :ncn], op=OP.subtract)
                    e_m = gsb.tile([128, 512], F16, tag="g")
                    nc.vector.tensor_tensor(out=e_m[:, :ncn], in0=z_sb[m][:, :ncn],
                                            in1=d_m[:, :ncn], op=OP.mult)
                    nc.vector.tensor_tensor(out=hnews[m][:, n0:n1], in0=n_sb[m][:, :ncn],
                                            in1=e_m[:, :ncn], op=OP.add)
                a_ps = ps8.tile([1, 512], F32, tag="agg", bufs=5, name=f"aps{ci}")
                nc.tensor.matmul(a_ps[:, :ncn], lhsT=gawx_s[:],
                                 rhs=xt_s[:, n0:n1], start=True, stop=False,
                                 skip_group_check=True)
                for m in range(2):
                    g_ps = ps8.tile([128, 512], F32, tag="agg", bufs=5, name=f"gps{ci}{m}")
                    for k in range(2):
                        nc.tensor.matmul(
                            g_ps[:, :ncn], lhsT=hgw_s[:, 256 * k + 128 * m:256 * k + 128 * m + 128],
                            rhs=hnews[k][:, n0:n1], start=(k == 0), stop=(k == 1))
                    g_m = gsb.tile([128, 512], F16, tag="g")
                    nc.vector.tensor_scalar(
                        out=g_m[:, :ncn], in0=g_ps[:, :ncn],
                        scalar1=hgb_s[:, m:m + 1], scalar2=0.0,
                        op0=OP.add, op1=OP.max)
                    nc.tensor.matmul(a_ps[:, :ncn], lhsT=gawg_s[:, m:m + 1],
                                     rhs=g_m[:, :ncn], start=False,
                                     stop=(m == 1), skip_group_check=True)
                nc.scalar.activation(a_s[:, n0:n1], a_ps[:, :ncn],
                                     AF.Identity, bias=gab_s[:])

            # ---- edge phase (GRU chunks interleaved as agg banks free) ----
            CHUNK_BLOCKS = 8  # 16 tiles per DMA
            xg_chunk = None
            ohb = None
            pair_ctr = 0
            pending_scatters = None
            for j in range(NBLK):
                for t in range(T_w):
                    qi = (j * T_w + t) * 4
                    if qi % 16 == 0:
                        if qi == 0:
                            xg_chunk = xg_first
                        else:
                            c0 = (qi // 2) * 128
                            csz = min(CHUNK_BLOCKS * 128, xg_cols - c0)
                            xg_chunk = xg_pool.tile([128, CHUNK_BLOCKS * 128], F16)
                            nc.sync.dma_start(xg_chunk[:, :csz], d_xg[:, c0:c0 + csz])
                    if qi % 8 == 0:
                        ohb = oh_pool.tile([128, 256], F16)
                        nc.vector.tensor_tensor(
                            out=ohb[:].rearrange("p (a b) -> p a b", b=32),
                            in0=iota_s[:].rearrange("p (a b) -> p a b", b=32),
                            in1=offs_s[:, qi:qi + 8].to_broadcast([128, 8, 32]),
                            op=OP.is_equal)
                    # block-diag lin1: one matmul computes a pair of tiles
                    # (K=66 stacked xg, N=512 block-diagonal weights)
                    mps = []
                    for pr in range(2):
                        mp = ps8.tile([128, 512], F32, tag="mp", bufs=3)
                        blk = (qi >> 1) + pr
                        ccol = 128 * (blk % CHUNK_BLOCKS)
                        nc.tensor.matmul(
                            mp[:], lhsT=xg_chunk[0:66, ccol:ccol + 128],
                            rhs=l1w_s[:], start=True, stop=True)
                        ms = msg_pool.tile([128, 512], F16, tag="ms")
                        if pair_ctr % 9 < 5:
                            nc.scalar.activation(ms[:], mp[:], AF.Relu)
                        else:
                            nc.vector.tensor_scalar_max(ms[:], mp[:], 0.0)
                        pair_ctr += 1
                        mps.append(ms)
                    last = (j == NBLK - 1 and t == T_w - 1)

                    def emit_scatters(j=j, qi=qi, mps=mps, ohb=ohb, last=last):
                        for q in range(4):
                            idx = qi + q
                            g = 4 * j + q
                            for hb in range(2):
                                col = 64 * g + 32 * hb
                                nc.tensor.matmul(
                                    agg_b[col // 512][:, col % 512:col % 512 + 32],
                                    lhsT=mps[q // 2][:, 256 * (q % 2) + 128 * hb:256 * (q % 2) + 128 * hb + 128],
                                    rhs=ohb[:, 32 * (idx % 8):32 * (idx % 8) + 32],
                                    start=False, stop=last,
                                    skip_group_check=True)
                    # pipeline: emit previous quad's scatters after this
                    # quad's lin1 so the relu latency hides behind PE work
                    if pending_scatters is not None:
                        pending_scatters()
                    pending_scatters = emit_scatters

                if j % 2 == 1:
                    if pending_scatters is not None:  # bank must be complete
                        pending_scatters()
                        pending_scatters = None
                    # bank j//2 complete: evacuate interleaved halves to aggT
                    src_v = agg_b[j // 2][:].rearrange("p (w t c) -> p w t c",
                                                       t=2, c=32)
                    for hb in range(2):
                        dst = aggT[hb][:, 128 * (j - 1):128 * (j - 1) + 256]
                        dst_v = dst.rearrange("p (w c) -> p w c", c=32)
                        if hb == 0:
                            nc.scalar.activation(dst_v, src_v[:, :, 0, :], AF.Copy)
                        else:
                            nc.vector.tensor_copy(dst_v, src_v[:, :, 1, :])
                if j == 3:
                    emit_gru_chunk(0)
                elif j == 7:
                    emit_gru_chunk(1)

            emit_gru_chunk(2)
            nc.sync.dma_start(
                d_hout[:].rearrange("(k p) n -> p k n", p=128),
                hnew_s[:].rearrange("p (k n) -> p k n", k=2))
            nc.sync.dma_start(d_aout[:], a_s[:])

    nc.compile()
    return nc


def _shared_inputs(inputs):
    lin1_w = inputs["lin1_w"].astype(np.float32)
    lin1_b = inputs["lin1_b"].astype(np.float32)
    w_ih = inputs["w_ih"].astype(np.float32)
    w_hh = inputs["w_hh"].astype(np.float32)
    b_ih = inputs["b_ih"].astype(np.float32)
    b_hh = inputs["b_hh"].astype(np.float32)
    hg_w = inputs["hg_w"].astype(np.float32)
    hg_b = inputs["hg_b"].astype(np.float32)
    ga_w = inputs["ga_w"].astype(np.float32)
    ga_b = inputs["ga_b"].astype(np.float32)

    l1w = np.zeros((66, 512), dtype=np.float32)
    l1w[:32, :256] = lin1_w.T
    l1w[32, :256] = lin1_b
    l1w[33:65, 256:] = lin1_w.T
    l1w[65, 256:] = lin1_b
    wihT = np.ascontiguousarray(w_ih.T)
    whhT = np.ascontiguousarray(w_hh.T)
    f16 = lambda a: np.ascontiguousarray(np.asarray(a, np.float32).astype(NP16))
    f32 = lambda a: np.ascontiguousarray(np.asarray(a, np.float32))
    return {
        "iota": f16(np.tile(np.tile(np.arange(32, dtype=np.float32), 8), (128, 1))),
        "l1w": f16(l1w),
        "wrz": f16(np.concatenate([wihT[:, :512], whhT[:, :512]], axis=0)),
        "win": f16(wihT[:, 512:]),
        "whn": f16(whhT[:, 512:]),
        "hgw": f16(hg_w.T),
        "gawx": f16(ga_w[:, :32].T),
        "gawg": f16(ga_w[:, 32:].T),
        "brz": f32((b_ih[:512] + b_hh[:512]).reshape(512, 1)),
        "bin": f32(b_ih[512:].reshape(256, 1)),
        "bhn": f32(b_hh[512:].reshape(256, 1)),
        "hgb": f32(hg_b.reshape(256, 1)),
        "gab": f32(ga_b.reshape(1, 1)),
    }


def _core_inputs(inputs, T_w, cores):
    x = inputs["x"].astype(np.float32)
    h = inputs["h"].astype(np.float32)
    shared = _shared_inputs(inputs)
    in_maps = []
    for c in range(N_CORES):
        lo = c * NODES_PER_CORE
        hi = min(lo + NODES_PER_CORE, N_NODES)
        ht = np.zeros((256, NODES_PER_CORE), dtype=np.float32)
        ht[:, :hi - lo] = h[lo:hi].T
        xt = np.zeros((32, NODES_PER_CORE), dtype=np.float32)
        xt[:, :hi - lo] = x[lo:hi].T
        m = dict(shared)
        m["xg"] = cores[c]["xg"]
        m["offs"] = cores[c]["offs"]
        m["ht"] = np.ascontiguousarray(ht.astype(NP16))
        m["xt"] = np.ascontiguousarray(xt.astype(NP16))
        in_maps.append(m)
    return in_maps


def kernel(**inputs):
    x = inputs["x"].astype(np.float32)
    T_w, cores = _preprocess(x, inputs["edge_index"])
    if T_w not in _CACHE:
        _CACHE[T_w] = _build_graph(T_w)
    nc = _CACHE[T_w]
    in_maps = _core_inputs(inputs, T_w, cores)
    res = run_bass_kernel_spmd(nc, in_maps, core_ids=list(range(N_CORES)))

    h_new = np.empty((N_NODES, H), dtype=np.float32)
    a = np.empty((N_NODES, 1), dtype=np.float32)
    for c in range(N_CORES):
        lo = c * NODES_PER_CORE
        hi = min(lo + NODES_PER_CORE, N_NODES)
        h_new[lo:hi] = res.results[c]["h_out"][:, :hi - lo].T.astype(np.float32)
        a[lo:hi, 0] = res.results[c]["a_out"][0, :hi - lo]
    a = np.log1p(np.exp(-np.abs(a))) + np.maximum(a, 0.0)  # softplus
    return (a, h_new)


# revision 40
# speedup vs baseline: 1.1355x; 1.1355x over previous
"""Trainium2 Bass kernel for GNN message passing + GRU + MLP head.

Strategy:
  - Sort edges by destination on host; nodes split into 8 ranges of 1280,
    one per NeuronCore -> no collectives.
  - Edges packed into 128-edge tiles grouped by 32-node destination
    windows; windows processed in quads (4 PSUM col-strips) so scatter
    matmuls overlap via tile_position col groups.
  - All matmuls in fp16 (fp32 matmul runs 2-pass LOW_HIGH on TRN2; fp16
    is single-pass with enough mantissa for ~1e-3 end-to-end error).
  - Per tile: lin1 matmul (K=33 incl bias row) -> relu (ACT/DVE split,
    f32 PSUM -> fp16 SBUF) -> one-hot scatter matmul accumulated into
    PSUM-resident agg (has_written cleared by K=1 dummy matmuls).
  - agg transposed on-chip (PE transpose) to feed GRU matmuls; GRU gates
    + MLP head in [H, nodes] layout; outputs un-transposed on host.
"""

import numpy as np
import ml_dtypes

import concourse.bass as bass
import concourse.tile as tile
from concourse import bacc, mybir
from concourse.bass_utils import run_bass_kernel_spmd
from concourse.masks import make_identity

N_NODES = 10000
N_EDGES = 320000
D_IN = 32
H = 256
N_CORES = 8
WIN = 32
NODES_PER_CORE = 1280
N_WIN = NODES_PER_CORE // WIN    # 40 windows/core
NBLK = N_WIN // 4                # 10 window quads
P = 128
F32 = mybir.dt.float32
F16 = mybir.dt.float16
NP16 = np.float16
BF16 = mybir.dt.bfloat16
NPBF = ml_dtypes.bfloat16

_CACHE = {}


def _preprocess(x, edge_index):
    """Sort/partition edges; build per-core packed fp16 inputs."""
    row = np.asarray(edge_index[0], dtype=np.int64)
    col = np.asarray(edge_index[1], dtype=np.int64)
    order = np.argsort(col, kind="stable")
    col_s = col[order]
    row_s = row[order]

    n_win_glob = N_CORES * N_WIN
    bounds = np.searchsorted(col_s, np.arange(0, n_win_glob + 1) * WIN)
    cnt = bounds[1:] - bounds[:-1]
    T_w = max(1, int(np.max((cnt + P - 1) // P)))

    n_tiles = N_WIN * T_w
    e_slot = n_tiles * P
    cores = []
    for c in range(N_CORES):
        src = np.full(e_slot, -1, dtype=np.int64)
        offs = np.full(e_slot, 64.0, dtype=np.float32)
        # tile order: j (quad) -> t (slot) -> q (window in quad)
        idx = 0
        for j in range(NBLK):
            for t in range(T_w):
                for q in range(4):
                    g = 4 * j + q
                    w = c * N_WIN + g
                    lo, hi = bounds[w], bounds[w + 1]
                    s0 = lo + t * P
                    s1 = min(s0 + P, hi)
                    k = s1 - s0
                    if k > 0:
                        b = idx * P
                        src[b:b + k] = row_s[s0:s1]
                        offs[b:b + k] = (col_s[s0:s1] - w * WIN).astype(np.float32)
                    idx += 1
        valid = src >= 0
        xg = np.zeros((e_slot, 33), dtype=np.float32)
        xg[valid, :D_IN] = x[src[valid]]
        xg[:, D_IN] = 1.0  # bias row
        t3 = xg.reshape(n_tiles, P, 33).transpose(0, 2, 1)  # [T, 33, 128]
        pack = np.zeros((n_tiles // 2, 128, 128), dtype=np.float32)
        pack[:, 0:33, :] = t3[0::2]
        pack[:, 33:66, :] = t3[1::2]
        xg_pack = pack.transpose(1, 0, 2).reshape(128, (n_tiles // 2) * 128)
        offs_arr = offs.reshape(n_tiles, P).T  # [128, T]
        cores.append({
            "xg": np.ascontiguousarray(xg_pack.astype(NP16)),
            "offs": np.ascontiguousarray(offs_arr.astype(NP16)),
        })
    return T_w, cores


def _build_graph(T_w):
    n_tiles = N_WIN * T_w
    xg_cols = (n_tiles // 2) * 128
    NPAD = NODES_PER_CORE

    nc = bacc.Bacc()
    d_xg = nc.declare_dram_parameter("xg", [128, xg_cols], F16, isOutput=False)
    d_offs = nc.declare_dram_parameter("offs", [128, n_tiles], F16, isOutput=False)
    d_iota = nc.declare_dram_parameter("iota", [128, 256], F16, isOutput=False)
    d_l1w = nc.declare_dram_parameter("l1w", [66, 512], F16, isOutput=False)
    d_ht = nc.declare_dram_parameter("ht", [256, NPAD], F16, isOutput=False)
    d_xt = nc.declare_dram_parameter("xt", [32, NPAD], F16, isOutput=False)
    d_wrz = nc.declare_dram_parameter("wrz", [512, 512], F16, isOutput=False)
    d_win = nc.declare_dram_parameter("win", [256, 256], F16, isOutput=False)
    d_whn = nc.declare_dram_parameter("whn", [256, 256], F16, isOutput=False)
    d_hgw = nc.declare_dram_parameter("hgw", [256, 256], F16, isOutput=False)
    d_gawx = nc.declare_dram_parameter("gawx", [32, 1], F16, isOutput=False)
    d_gawg = nc.declare_dram_parameter("gawg", [256, 1], F16, isOutput=False)
    d_brz = nc.declare_dram_parameter("brz", [512, 1], F32, isOutput=False)
    d_bin = nc.declare_dram_parameter("bin", [256, 1], F32, isOutput=False)
    d_bhn = nc.declare_dram_parameter("bhn", [256, 1], F32, isOutput=False)
    d_hgb = nc.declare_dram_parameter("hgb", [256, 1], F32, isOutput=False)
    d_gab = nc.declare_dram_parameter("gab", [1, 1], F32, isOutput=False)
    d_hout = nc.declare_dram_parameter("h_out", [256, NPAD], F16, isOutput=True)
    d_aout = nc.declare_dram_parameter("a_out", [1, NPAD], F32, isOutput=True)

    AF = mybir.ActivationFunctionType
    OP = mybir.AluOpType

    with tile.TileContext(nc) as tc:
        with (
            tc.tile_pool(name="const", bufs=1) as cpool,
            tc.tile_pool(name="ps8", bufs=1, space="PSUM") as ps8,
            tc.tile_pool(name="xgc", bufs=3) as xg_pool,
            tc.tile_pool(name="ohb", bufs=4) as oh_pool,
            tc.tile_pool(name="msgs", bufs=6) as msg_pool,
            tc.tile_pool(name="gsb", bufs=14) as gsb,
        ):
            # ---- first xg tiles first: minimize PE start latency ----
            xg_first = xg_pool.tile([128, 8 * 128], F16, tag="xgf", bufs=1)
            for pc in range(4):
                nc.sync.dma_start(xg_first[:, 256 * pc:256 * pc + 256],
                                  d_xg[:, 256 * pc:256 * pc + 256])
            # ---- constants / inputs (sync queue: edge data first) ----
            iota_s = cpool.tile([128, 256], F16)
            nc.sync.dma_start(iota_s[:], d_iota[:])
            l1w_s = cpool.tile([66, 512], F16)  # block-diag for tile pairs
            nc.sync.dma_start(l1w_s[:], d_l1w[:])
            offs_s = cpool.tile([128, n_tiles], F16)
            nc.sync.dma_start(offs_s[:], d_offs[:])
            zc = cpool.tile([1, 128], F16)
            nc.vector.memset(zc[:], 0.0)
            zr = cpool.tile([1, 512], F16)
            nc.vector.memset(zr[:], 0.0)

            # ---- GRU weights / node inputs (gpsimd queue) ----
            ht_s = cpool.tile([128, 2 * NPAD], F16)
            nc.gpsimd.dma_start(
                ht_s[:].rearrange("p (k n) -> p k n", k=2),
                d_ht[:].rearrange("(k p) n -> p k n", p=128))
            xt_s = cpool.tile([32, NPAD], F16)
            nc.gpsimd.dma_start(xt_s[:], d_xt[:])
            wrz_s = cpool.tile([128, 2048], F16)
            nc.gpsimd.dma_start(
                wrz_s[:].rearrange("p (k m) -> p k m", k=4),
                d_wrz[:].rearrange("(k p) m -> p k m", p=128))
            win_s = cpool.tile([128, 512], F16)
            nc.gpsimd.dma_start(
                win_s[:].rearrange("p (k m) -> p k m", k=2),
                d_win[:].rearrange("(k p) m -> p k m", p=128))
            whn_s = cpool.tile([128, 512], F16)
            nc.gpsimd.dma_start(
                whn_s[:].rearrange("p (k m) -> p k m", k=2),
                d_whn[:].rearrange("(k p) m -> p k m", p=128))
            hgw_s = cpool.tile([128, 512], F16)
            nc.gpsimd.dma_start(
                hgw_s[:].rearrange("p (k m) -> p k m", k=2),
                d_hgw[:].rearrange("(k p) m -> p k m", p=128))
            gawx_s = cpool.tile([32, 1], F16)
            nc.gpsimd.dma_start(gawx_s[:], d_gawx[:])
            gawg_s = cpool.tile([128, 2], F16)
            nc.gpsimd.dma_start(
                gawg_s[:].rearrange("p (k o) -> p k o", k=2),
                d_gawg[:].rearrange("(k p) o -> p k o", p=128))
            brz_s = cpool.tile([128, 4], F32)
            nc.gpsimd.dma_start(
                brz_s[:].rearrange("p (k o) -> p k o", k=4),
                d_brz[:].rearrange("(k p) o -> p k o", p=128))
            bin_s = cpool.tile([128, 2], F32)
            nc.gpsimd.dma_start(
                bin_s[:].rearrange("p (k o) -> p k o", k=2),
                d_bin[:].rearrange("(k p) o -> p k o", p=128))
            bhn_s = cpool.tile([128, 2], F32)
            nc.gpsimd.dma_start(
                bhn_s[:].rearrange("p (k o) -> p k o", k=2),
                d_bhn[:].rearrange("(k p) o -> p k o", p=128))
            hgb_s = cpool.tile([128, 2], F32)
            nc.gpsimd.dma_start(
                hgb_s[:].rearrange("p (k o) -> p k o", k=2),
                d_hgb[:].rearrange("(k p) o -> p k o", p=128))
            gab_s = cpool.tile([1, 1], F32)
            nc.gpsimd.dma_start(gab_s[:], d_gab[:])

            # ---- persistent SBUF state ----
            aggT = [cpool.tile([128, NPAD], F16, name=f"aggT{i}") for i in range(2)]
            hts = [ht_s[:, :NPAD], ht_s[:, NPAD:]]
            hnew_s = cpool.tile([128, 2 * NPAD], F16)
            hnews = [hnew_s[:, :NPAD], hnew_s[:, NPAD:]]
            a_s = cpool.tile([1, NPAD], F32)

            # ---- agg PSUM banks (slots recycle into GRU psum) ----
            agg_b = [ps8.tile([128, 512], F32, tag="agg", bufs=5, name=f"agg_b{k}")
                     for k in range(5)]
            for k in range(5):  # clear has_written bits
                nc.tensor.matmul(agg_b[k][:], lhsT=zc[:], rhs=zr[:],
                                 start=True, stop=False, skip_group_check=True)

            kstack = [aggT[0][:], aggT[1][:], hts[0], hts[1]]
            NCHUNK = [(0, 512), (512, 1024), (1024, NPAD)]

            def emit_gru_chunk(ci):
                n0, n1 = NCHUNK[ci]
                ncn = n1 - n0
                rz_ps = []
                for m in range(4):
                    ps = ps8.tile([128, 512], F32, tag="agg", bufs=5, name=f"rz{ci}{m}")
                    for k in (2, 3, 0, 1):  # h-side first
                        nc.tensor.matmul(
                            ps[:, :ncn], lhsT=wrz_s[:, 512 * k + 128 * m:512 * k + 128 * m + 128],
                            rhs=kstack[k][:, n0:n1], start=(k == 2), stop=(k == 1))
                    rz_ps.append(ps)
                hn_ps = []
                for m in range(2):
                    ps = ps8.tile([128, 512], F32, tag="agg", bufs=5, name=f"hn{ci}{m}")
                    for k in range(2):
                        nc.tensor.matmul(
                            ps[:, :ncn], lhsT=whn_s[:, 256 * k + 128 * m:256 * k + 128 * m + 128],
                            rhs=hts[k][:, n0:n1], start=(k == 0), stop=(k == 1))
                    hn_ps.append(ps)
                in_ps = []
                for m in range(2):
                    ps = ps8.tile([128, 512], F32, tag="agg", bufs=5, name=f"in{ci}{m}")
                    for k in range(2):
                        nc.tensor.matmul(
                            ps[:, :ncn], lhsT=win_s[:, 256 * k + 128 * m:256 * k + 128 * m + 128],
                            rhs=aggT[k][:, n0:n1], start=(k == 0), stop=(k == 1))
                    in_ps.append(ps)

                n_sb = []
                z_sb = []
                for m in range(2):
                    r_m = gsb.tile([128, 512], F16, tag="g")
                    nc.scalar.activation(r_m[:, :ncn], rz_ps[m][:, :ncn],
                                         AF.Sigmoid, bias=brz_s[:, m:m + 1])
                    z_m = gsb.tile([128, 512], F16, tag="g")
                    nc.scalar.activation(z_m[:, :ncn], rz_ps[2 + m][:, :ncn],
                                         AF.Sigmoid, bias=brz_s[:, 2 + m:3 + m])
                    z_sb.append(z_m)
                    t1 = gsb.tile([128, 512], F16, tag="g")
                    nc.vector.scalar_tensor_tensor(
                        out=t1[:, :ncn], in0=hn_ps[m][:, :ncn],
                        scalar=bhn_s[:, m:m + 1], in1=r_m[:, :ncn],
                        op0=OP.add, op1=OP.mult)
                    t2 = gsb.tile([128, 512], F16, tag="g")
                    nc.vector.tensor_tensor(out=t2[:, :ncn], in0=t1[:, :ncn],
                                            in1=in_ps[m][:, :ncn], op=OP.add)
                    n_m = gsb.tile([128, 512], F16, tag="g")
                    nc.scalar.activation(n_m[:, :ncn], t2[:, :ncn],
                                         AF.Tanh, bias=bin_s[:, m:m + 1])
                    n_sb.append(n_m)
                for m in range(2):
                    d_m = gsb.tile([128, 512], F16, tag="g")
                    nc.vector.tensor_tensor(out=d_m[:, :ncn], in0=hts[m][:, n0:n1],
                                            in1=n_sb[m][:, :ncn], op=OP.subtract)
                    e_m = gsb.tile([128, 512], F16, tag="g")
                    nc.vector.tensor_tensor(out=e_m[:, :ncn], in0=z_sb[m][:, :ncn],
                                            in1=d_m[:, :ncn], op=OP.mult)
                    nc.vector.tensor_tensor(out=hnews[m][:, n0:n1], in0=n_sb[m][:, :ncn],
                                            in1=e_m[:, :ncn], op=OP.add)
                a_ps = ps8.tile([1, 512], F32, tag="agg", bufs=5, name=f"aps{ci}")
                nc.tensor.matmul(a_ps[:, :ncn], lhsT=gawx_s[:],
                                 rhs=xt_s[:, n0:n1], start=True, stop=False,
                                 skip_group_check=True)
                for m in range(2):
                    g_ps = ps8.tile([128, 512], F32, tag="agg", bufs=5, name=f"gps{ci}{m}")
                    for k in range(2):
                        nc.tensor.matmul(
                            g_ps[:, :ncn], lhsT=hgw_s[:, 256 * k + 128 * m:256 * k + 128 * m + 128],
                            rhs=hnews[k][:, n0:n1], start=(k == 0), stop=(k == 1))
                    g_m = gsb.tile([128, 512], F16, tag="g")
                    nc.vector.tensor_scalar(
                        out=g_m[:, :ncn], in0=g_ps[:, :ncn],
                        scalar1=hgb_s[:, m:m + 1], scalar2=0.0,
                        op0=OP.add, op1=OP.max)
                    nc.tensor.matmul(a_ps[:, :ncn], lhsT=gawg_s[:, m:m + 1],
                                     rhs=g_m[:, :ncn], start=False,
                                     stop=(m == 1), skip_group_check=True)
                nc.scalar.activation(a_s[:, n0:n1], a_ps[:, :ncn],
                                     AF.Identity, bias=gab_s[:])

            # ---- edge phase (GRU chunks interleaved as agg banks free) ----
            CHUNK_BLOCKS = 8  # 16 tiles per DMA
            xg_chunk = None
            ohb = None
            pair_ctr = 0
            for j in range(NBLK):
                for t in range(T_w):
                    qi = (j * T_w + t) * 4
                    if qi % 16 == 0:
                        if qi == 0:
                            xg_chunk = xg_first
                        else:
                            c0 = (qi // 2) * 128
                            csz = min(CHUNK_BLOCKS * 128, xg_cols - c0)
                            xg_chunk = xg_pool.tile([128, CHUNK_BLOCKS * 128], F16)
                            nc.sync.dma_start(xg_chunk[:, :csz], d_xg[:, c0:c0 + csz])
                    if qi % 8 == 0:
                        ohb = oh_pool.tile([128, 256], F16)
                        nc.vector.tensor_tensor(
                            out=ohb[:].rearrange("p (a b) -> p a b", b=32),
                            in0=iota_s[:].rearrange("p (a b) -> p a b", b=32),
                            in1=offs_s[:, qi:qi + 8].to_broadcast([128, 8, 32]),
                            op=OP.is_equal)
                    # block-diag lin1: one matmul computes a pair of tiles
                    # (K=66 stacked xg, N=512 block-diagonal weights)
                    mps = []
                    for pr in range(2):
                        mp = ps8.tile([128, 512], F32, tag="mp", bufs=3)
                        blk = (qi >> 1) + pr
                        ccol = 128 * (blk % CHUNK_BLOCKS)
                        nc.tensor.matmul(
                            mp[:], lhsT=xg_chunk[0:66, ccol:ccol + 128],
                            rhs=l1w_s[:], start=True, stop=True)
                        ms = msg_pool.tile([128, 512], F16, tag="ms")
                        if pair_ctr % 9 < 5:
                            nc.scalar.activation(ms[:], mp[:], AF.Relu)
                        else:
                            nc.vector.tensor_scalar_max(ms[:], mp[:], 0.0)
                        pair_ctr += 1
                        mps.append(ms)
                    last = (j == NBLK - 1 and t == T_w - 1)
                    for q in range(4):
                        idx = qi + q
                        g = 4 * j + q
                        for hb in range(2):
                            col = 64 * g + 32 * hb
                            nc.tensor.matmul(
                                agg_b[col // 512][:, col % 512:col % 512 + 32],
                                lhsT=mps[q // 2][:, 256 * (q % 2) + 128 * hb:256 * (q % 2) + 128 * hb + 128],
                                rhs=ohb[:, 32 * (idx % 8):32 * (idx % 8) + 32],
                                start=False, stop=last,
                                skip_group_check=True)

                if j % 2 == 1:
                    # bank j//2 complete: evacuate interleaved halves to aggT
                    src_v = agg_b[j // 2][:].rearrange("p (w t c) -> p w t c",
                                                       t=2, c=32)
                    for hb in range(2):
                        dst = aggT[hb][:, 128 * (j - 1):128 * (j - 1) + 256]
                        dst_v = dst.rearrange("p (w c) -> p w c", c=32)
                        if hb == 0:
                            nc.scalar.activation(dst_v, src_v[:, :, 0, :], AF.Copy)
                        else:
                            nc.vector.tensor_copy(dst_v, src_v[:, :, 1, :])
                if j == 3:
                    emit_gru_chunk(0)
                elif j == 7:
                    emit_gru_chunk(1)

            emit_gru_chunk(2)
            nc.sync.dma_start(
                d_hout[:].rearrange("(k p) n -> p k n", p=128),
                hnew_s[:].rearrange("p (k n) -> p k n", k=2))
            nc.sync.dma_start(d_aout[:], a_s[:])

    nc.compile()
    return nc


def _shared_inputs(inputs):
    lin1_w = inputs["lin1_w"].astype(np.float32)
    lin1_b = inputs["lin1_b"].astype(np.float32)
    w_ih = inputs["w_ih"].astype(np.float32)
    w_hh = inputs["w_hh"].astype(np.float32)
    b_ih = inputs["b_ih"].astype(np.float32)
    b_hh = inputs["b_hh"].astype(np.float32)
    hg_w = inputs["hg_w"].astype(np.float32)
    hg_b = inputs["hg_b"].astype(np.float32)
    ga_w = inputs["ga_w"].astype(np.float32)
    ga_b = inputs["ga_b"].astype(np.float32)

    l1w = np.zeros((66, 512), dtype=np.float32)
    l1w[:32, :256] = lin1_w.T
    l1w[32, :256] = lin1_b
    l1w[33:65, 256:] = lin1_w.T
    l1w[65, 256:] = lin1_b
    wihT = np.ascontiguousarray(w_ih.T)
    whhT = np.ascontiguousarray(w_hh.T)
    f16 = lambda a: np.ascontiguousarray(np.asarray(a, np.float32).astype(NP16))
    f32 = lambda a: np.ascontiguousarray(np.asarray(a, np.float32))
    return {
        "iota": f16(np.tile(np.tile(np.arange(32, dtype=np.float32), 8), (128, 1))),
        "l1w": f16(l1w),
        "wrz": f16(np.concatenate([wihT[:, :512], whhT[:, :512]], axis=0)),
        "win": f16(wihT[:, 512:]),
        "whn": f16(whhT[:, 512:]),
        "hgw": f16(hg_w.T),
        "gawx": f16(ga_w[:, :32].T),
        "gawg": f16(ga_w[:, 32:].T),
        "brz": f32((b_ih[:512] + b_hh[:512]).reshape(512, 1)),
        "bin": f32(b_ih[512:].reshape(256, 1)),
        "bhn": f32(b_hh[512:].reshape(256, 1)),
        "hgb": f32(hg_b.reshape(256, 1)),
        "gab": f32(ga_b.reshape(1, 1)),
    }


def _core_inputs(inputs, T_w, cores):
    x = inputs["x"].astype(np.float32)
    h = inputs["h"].astype(np.float32)
    shared = _shared_inputs(inputs)
    in_maps = []
    for c in range(N_CORES):
        lo = c * NODES_PER_CORE
        hi = min(lo + NODES_PER_CORE, N_NODES)
        ht = np.zeros((256, NODES_PER_CORE), dtype=np.float32)
        ht[:, :hi - lo] = h[lo:hi].T
        xt = np.zeros((32, NODES_PER_CORE), dtype=np.float32)
        xt[:, :hi - lo] = x[lo:hi].T
        m = dict(shared)
        m["xg"] = cores[c]["xg"]
        m["offs"] = cores[c]["offs"]
        m["ht"] = np.ascontiguousarray(ht.astype(NP16))
        m["xt"] = np.ascontiguousarray(xt.astype(NP16))
        in_maps.append(m)
    return in_maps


def kernel(**inputs):
    x = inputs["x"].astype(np.float32)
    T_w, cores = _preprocess(x, inputs["edge_index"])
    if T_w not in _CACHE:
        _CACHE[T_w] = _build_graph(T_w)
    nc = _CACHE[T_w]
    in_maps = _core_inputs(inputs, T_w, cores)
    res = run_bass_kernel_spmd(nc, in_maps, core_ids=list(range(N_CORES)))

    h_new = np.empty((N_NODES, H), dtype=np.float32)
    a = np.empty((N_NODES, 1), dtype=np.float32)
    for c in range(N_CORES):
        lo = c * NODES_PER_CORE
        hi = min(lo + NODES_PER_CORE, N_NODES)
        h_new[lo:hi] = res.results[c]["h_out"][:, :hi - lo].T.astype(np.float32)
        a[lo:hi, 0] = res.results[c]["a_out"][0, :hi - lo]
    a = np.log1p(np.exp(-np.abs(a))) + np.maximum(a, 0.0)  # softplus
    return (a, h_new)


# revision 41
# speedup vs baseline: 1.1592x; 1.0209x over previous
"""Trainium2 Bass kernel for GNN message passing + GRU + MLP head.

Strategy:
  - Sort edges by destination on host; nodes split into 8 ranges of 1280,
    one per NeuronCore -> no collectives.
  - Edges packed into 128-edge tiles grouped by 32-node destination
    windows; windows processed in quads (4 PSUM col-strips) so scatter
    matmuls overlap via tile_position col groups.
  - All matmuls in fp16 (fp32 matmul runs 2-pass LOW_HIGH on TRN2; fp16
    is single-pass with enough mantissa for ~1e-3 end-to-end error).
  - Per tile: lin1 matmul (K=33 incl bias row) -> relu (ACT/DVE split,
    f32 PSUM -> fp16 SBUF) -> one-hot scatter matmul accumulated into
    PSUM-resident agg (has_written cleared by K=1 dummy matmuls).
  - agg transposed on-chip (PE transpose) to feed GRU matmuls; GRU gates
    + MLP head in [H, nodes] layout; outputs un-transposed on host.
"""

import numpy as np
import ml_dtypes

import concourse.bass as bass
import concourse.tile as tile
from concourse import bacc, mybir
from concourse.bass_utils import run_bass_kernel_spmd
from concourse.masks import make_identity

N_NODES = 10000
N_EDGES = 320000
D_IN = 32
H = 256
N_CORES = 8
WIN = 32
NODES_PER_CORE = 1280
N_WIN = NODES_PER_CORE // WIN    # 40 windows/core
NBLK = N_WIN // 4                # 10 window quads
P = 128
F32 = mybir.dt.float32
F16 = mybir.dt.float16
NP16 = np.float16
BF16 = mybir.dt.bfloat16
NPBF = ml_dtypes.bfloat16

_CACHE = {}


def _preprocess(x, edge_index):
    """Sort/partition edges; build per-core packed fp16 inputs."""
    row = np.asarray(edge_index[0], dtype=np.int64)
    col = np.asarray(edge_index[1], dtype=np.int64)
    order = np.argsort(col, kind="stable")
    col_s = col[order]
    row_s = row[order]

    n_win_glob = N_CORES * N_WIN
    bounds = np.searchsorted(col_s, np.arange(0, n_win_glob + 1) * WIN)
    cnt = bounds[1:] - bounds[:-1]
    T_w = max(1, int(np.max((cnt + P - 1) // P)))

    n_tiles = N_WIN * T_w
    e_slot = n_tiles * P
    cores = []
    for c in range(N_CORES):
        src = np.full(e_slot, -1, dtype=np.int64)
        offs = np.full(e_slot, 64.0, dtype=np.float32)
        # tile order: j (quad) -> t (slot) -> q (window in quad)
        idx = 0
        for j in range(NBLK):
            for t in range(T_w):
                for q in range(4):
                    g = 4 * j + q
                    w = c * N_WIN + g
                    lo, hi = bounds[w], bounds[w + 1]
                    s0 = lo + t * P
                    s1 = min(s0 + P, hi)
                    k = s1 - s0
                    if k > 0:
                        b = idx * P
                        src[b:b + k] = row_s[s0:s1]
                        offs[b:b + k] = (col_s[s0:s1] - w * WIN).astype(np.float32)
                    idx += 1
        valid = src >= 0
        xg = np.zeros((e_slot, 33), dtype=np.float32)
        xg[valid, :D_IN] = x[src[valid]]
        xg[:, D_IN] = 1.0  # bias row
        t3 = xg.reshape(n_tiles, P, 33).transpose(0, 2, 1)  # [T, 33, 128]
        pack = np.zeros((n_tiles // 2, 128, 128), dtype=np.float32)
        pack[:, 0:33, :] = t3[0::2]
        pack[:, 33:66, :] = t3[1::2]
        xg_pack = pack.transpose(1, 0, 2).reshape(128, (n_tiles // 2) * 128)
        offs_arr = offs.reshape(n_tiles, P).T  # [128, T]
        cores.append({
            "xg": np.ascontiguousarray(xg_pack.astype(NP16)),
            "offs": np.ascontiguousarray(offs_arr.astype(NP16)),
        })
    return T_w, cores


def _build_graph(T_w):
    n_tiles = N_WIN * T_w
    xg_cols = (n_tiles // 2) * 128
    NPAD = NODES_PER_CORE

    nc = bacc.Bacc()
    d_xg = nc.declare_dram_parameter("xg", [128, xg_cols], F16, isOutput=False)
    d_offs = nc.declare_dram_parameter("offs", [128, n_tiles], F16, isOutput=False)
    d_iota = nc.declare_dram_parameter("iota", [128, 256], F16, isOutput=False)
    d_l1w = nc.declare_dram_parameter("l1w", [66, 512], F16, isOutput=False)
    d_ht = nc.declare_dram_parameter("ht", [256, NPAD], F16, isOutput=False)
    d_xt = nc.declare_dram_parameter("xt", [32, NPAD], F16, isOutput=False)
    d_wrz = nc.declare_dram_parameter("wrz", [512, 512], F16, isOutput=False)
    d_win = nc.declare_dram_parameter("win", [256, 256], F16, isOutput=False)
    d_whn = nc.declare_dram_parameter("whn", [256, 256], F16, isOutput=False)
    d_hgw = nc.declare_dram_parameter("hgw", [256, 256], F16, isOutput=False)
    d_gawx = nc.declare_dram_parameter("gawx", [32, 1], F16, isOutput=False)
    d_gawg = nc.declare_dram_parameter("gawg", [256, 1], F16, isOutput=False)
    d_brz = nc.declare_dram_parameter("brz", [512, 1], F32, isOutput=False)
    d_bin = nc.declare_dram_parameter("bin", [256, 1], F32, isOutput=False)
    d_bhn = nc.declare_dram_parameter("bhn", [256, 1], F32, isOutput=False)
    d_hgb = nc.declare_dram_parameter("hgb", [256, 1], F32, isOutput=False)
    d_gab = nc.declare_dram_parameter("gab", [1, 1], F32, isOutput=False)
    d_hout = nc.declare_dram_parameter("h_out", [256, NPAD], F16, isOutput=True)
    d_aout = nc.declare_dram_parameter("a_out", [1, NPAD], F32, isOutput=True)

    AF = mybir.ActivationFunctionType
    OP = mybir.AluOpType

    with tile.TileContext(nc) as tc:
        with (
            tc.tile_pool(name="const", bufs=1) as cpool,
            tc.tile_pool(name="ps8", bufs=1, space="PSUM") as ps8,
            tc.tile_pool(name="xgc", bufs=3) as xg_pool,
            tc.tile_pool(name="ohb", bufs=4) as oh_pool,
            tc.tile_pool(name="msgs", bufs=8) as msg_pool,
            tc.tile_pool(name="gsb", bufs=14) as gsb,
        ):
            # ---- first xg tiles first: minimize PE start latency ----
            xg_first = xg_pool.tile([128, 8 * 128], F16, tag="xgf", bufs=1)
            for pc in range(4):
                nc.sync.dma_start(xg_first[:, 256 * pc:256 * pc + 256],
                                  d_xg[:, 256 * pc:256 * pc + 256])
            # ---- constants / inputs (sync queue: edge data first) ----
            iota_s = cpool.tile([128, 256], F16)
            nc.sync.dma_start(iota_s[:], d_iota[:])
            l1w_s = cpool.tile([66, 512], F16)  # block-diag for tile pairs
            nc.sync.dma_start(l1w_s[:], d_l1w[:])
            offs_s = cpool.tile([128, n_tiles], F16)
            nc.sync.dma_start(offs_s[:], d_offs[:])
            zc = cpool.tile([1, 128], F16)
            nc.vector.memset(zc[:], 0.0)
            zr = cpool.tile([1, 512], F16)
            nc.vector.memset(zr[:], 0.0)

            # ---- GRU weights / node inputs (gpsimd queue) ----
            ht_s = cpool.tile([128, 2 * NPAD], F16)
            nc.gpsimd.dma_start(
                ht_s[:].rearrange("p (k n) -> p k n", k=2),
                d_ht[:].rearrange("(k p) n -> p k n", p=128))
            xt_s = cpool.tile([32, NPAD], F16)
            nc.gpsimd.dma_start(xt_s[:], d_xt[:])
            wrz_s = cpool.tile([128, 2048], F16)
            nc.gpsimd.dma_start(
                wrz_s[:].rearrange("p (k m) -> p k m", k=4),
                d_wrz[:].rearrange("(k p) m -> p k m", p=128))
            win_s = cpool.tile([128, 512], F16)
            nc.gpsimd.dma_start(
                win_s[:].rearrange("p (k m) -> p k m", k=2),
                d_win[:].rearrange("(k p) m -> p k m", p=128))
            whn_s = cpool.tile([128, 512], F16)
            nc.gpsimd.dma_start(
                whn_s[:].rearrange("p (k m) -> p k m", k=2),
                d_whn[:].rearrange("(k p) m -> p k m", p=128))
            hgw_s = cpool.tile([128, 512], F16)
            nc.gpsimd.dma_start(
                hgw_s[:].rearrange("p (k m) -> p k m", k=2),
                d_hgw[:].rearrange("(k p) m -> p k m", p=128))
            gawx_s = cpool.tile([32, 1], F16)
            nc.gpsimd.dma_start(gawx_s[:], d_gawx[:])
            gawg_s = cpool.tile([128, 2], F16)
            nc.gpsimd.dma_start(
                gawg_s[:].rearrange("p (k o) -> p k o", k=2),
                d_gawg[:].rearrange("(k p) o -> p k o", p=128))
            brz_s = cpool.tile([128, 4], F32)
            nc.gpsimd.dma_start(
                brz_s[:].rearrange("p (k o) -> p k o", k=4),
                d_brz[:].rearrange("(k p) o -> p k o", p=128))
            bin_s = cpool.tile([128, 2], F32)
            nc.gpsimd.dma_start(
                bin_s[:].rearrange("p (k o) -> p k o", k=2),
                d_bin[:].rearrange("(k p) o -> p k o", p=128))
            bhn_s = cpool.tile([128, 2], F32)
            nc.gpsimd.dma_start(
                bhn_s[:].rearrange("p (k o) -> p k o", k=2),
                d_bhn[:].rearrange("(k p) o -> p k o", p=128))
            hgb_s = cpool.tile([128, 2], F32)
            nc.gpsimd.dma_start(
                hgb_s[:].rearrange("p (k o) -> p k o", k=2),
                d_hgb[:].rearrange("(k p) o -> p k o", p=128))
            gab_s = cpool.tile([1, 1], F32)
            nc.gpsimd.dma_start(gab_s[:], d_gab[:])

            # ---- persistent SBUF state ----
            aggT = [cpool.tile([128, NPAD], F16, name=f"aggT{i}") for i in range(2)]
            hts = [ht_s[:, :NPAD], ht_s[:, NPAD:]]
            hnew_s = cpool.tile([128, 2 * NPAD], F16)
            hnews = [hnew_s[:, :NPAD], hnew_s[:, NPAD:]]
            a_s = cpool.tile([1, NPAD], F32)

            # ---- agg PSUM banks (slots recycle into GRU psum) ----
            agg_b = [ps8.tile([128, 512], F32, tag="agg", bufs=5, name=f"agg_b{k}")
                     for k in range(5)]
            for k in range(5):  # clear has_written bits
                nc.tensor.matmul(agg_b[k][:], lhsT=zc[:], rhs=zr[:],
                                 start=True, stop=False, skip_group_check=True)

            kstack = [aggT[0][:], aggT[1][:], hts[0], hts[1]]
            NCHUNK = [(0, 512), (512, 1024), (1024, NPAD)]

            def emit_gru_chunk(ci):
                n0, n1 = NCHUNK[ci]
                ncn = n1 - n0
                rz_ps = []
                for m in range(4):
                    ps = ps8.tile([128, 512], F32, tag="agg", bufs=5, name=f"rz{ci}{m}")
                    for k in (2, 3, 0, 1):  # h-side first
                        nc.tensor.matmul(
                            ps[:, :ncn], lhsT=wrz_s[:, 512 * k + 128 * m:512 * k + 128 * m + 128],
                            rhs=kstack[k][:, n0:n1], start=(k == 2), stop=(k == 1))
                    rz_ps.append(ps)
                hn_ps = []
                for m in range(2):
                    ps = ps8.tile([128, 512], F32, tag="agg", bufs=5, name=f"hn{ci}{m}")
                    for k in range(2):
                        nc.tensor.matmul(
                            ps[:, :ncn], lhsT=whn_s[:, 256 * k + 128 * m:256 * k + 128 * m + 128],
                            rhs=hts[k][:, n0:n1], start=(k == 0), stop=(k == 1))
                    hn_ps.append(ps)
                in_ps = []
                for m in range(2):
                    ps = ps8.tile([128, 512], F32, tag="agg", bufs=5, name=f"in{ci}{m}")
                    for k in range(2):
                        nc.tensor.matmul(
                            ps[:, :ncn], lhsT=win_s[:, 256 * k + 128 * m:256 * k + 128 * m + 128],
                            rhs=aggT[k][:, n0:n1], start=(k == 0), stop=(k == 1))
                    in_ps.append(ps)

                n_sb = []
                z_sb = []
                for m in range(2):
                    r_m = gsb.tile([128, 512], F16, tag="g")
                    nc.scalar.activation(r_m[:, :ncn], rz_ps[m][:, :ncn],
                                         AF.Sigmoid, bias=brz_s[:, m:m + 1])
                    z_m = gsb.tile([128, 512], F16, tag="g")
                    nc.scalar.activation(z_m[:, :ncn], rz_ps[2 + m][:, :ncn],
                                         AF.Sigmoid, bias=brz_s[:, 2 + m:3 + m])
                    z_sb.append(z_m)
                    t1 = gsb.tile([128, 512], F16, tag="g")
                    nc.vector.scalar_tensor_tensor(
                        out=t1[:, :ncn], in0=hn_ps[m][:, :ncn],
                        scalar=bhn_s[:, m:m + 1], in1=r_m[:, :ncn],
                        op0=OP.add, op1=OP.mult)
                    t2 = gsb.tile([128, 512], F16, tag="g")
                    nc.vector.tensor_tensor(out=t2[:, :ncn], in0=t1[:, :ncn],
                                            in1=in_ps[m][:, :ncn], op=OP.add)
                    n_m = gsb.tile([128, 512], F16, tag="g")
                    nc.scalar.activation(n_m[:, :ncn], t2[:, :ncn],
                                         AF.Tanh, bias=bin_s[:, m:m + 1])
                    n_sb.append(n_m)
                for m in range(2):
                    d_m = gsb.tile([128, 512], F16, tag="g")
                    nc.vector.tensor_tensor(out=d_m[:, :ncn], in0=hts[m][:, n0:n1],
                                            in1=n_sb[m][:, :ncn], op=OP.subtract)
                    e_m = gsb.tile([128, 512], F16, tag="g")
                    nc.vector.tensor_tensor(out=e_m[:, :ncn], in0=z_sb[m][:, :ncn],
                                            in1=d_m[:, :ncn], op=OP.mult)
                    nc.vector.tensor_tensor(out=hnews[m][:, n0:n1], in0=n_sb[m][:, :ncn],
                                            in1=e_m[:, :ncn], op=OP.add)
                a_ps = ps8.tile([1, 512], F32, tag="agg", bufs=5, name=f"aps{ci}")
                nc.tensor.matmul(a_ps[:, :ncn], lhsT=gawx_s[:],
                                 rhs=xt_s[:, n0:n1], start=True, stop=False,
                                 skip_group_check=True)
                for m in range(2):
                    g_ps = ps8.tile([128, 512], F32, tag="agg", bufs=5, name=f"gps{ci}{m}")
                    for k in range(2):
                        nc.tensor.matmul(
                            g_ps[:, :ncn], lhsT=hgw_s[:, 256 * k + 128 * m:256 * k + 128 * m + 128],
                            rhs=hnews[k][:, n0:n1], start=(k == 0), stop=(k == 1))
                    g_m = gsb.tile([128, 512], F16, tag="g")
                    nc.vector.tensor_scalar(
                        out=g_m[:, :ncn], in0=g_ps[:, :ncn],
                        scalar1=hgb_s[:, m:m + 1], scalar2=0.0,
                        op0=OP.add, op1=OP.max)
                    nc.tensor.matmul(a_ps[:, :ncn], lhsT=gawg_s[:, m:m + 1],
                                     rhs=g_m[:, :ncn], start=False,
                                     stop=(m == 1), skip_group_check=True)
                nc.scalar.activation(a_s[:, n0:n1], a_ps[:, :ncn],
                                     AF.Identity, bias=gab_s[:])

            # ---- edge phase (GRU chunks interleaved as agg banks free) ----
            CHUNK_BLOCKS = 8  # 16 tiles per DMA
            xg_chunk = None
            ohb = None
            pair_ctr = 0
            for j in range(NBLK):
                for t in range(T_w):
                    qi = (j * T_w + t) * 4
                    if qi % 16 == 0:
                        if qi == 0:
                            xg_chunk = xg_first
                        else:
                            c0 = (qi // 2) * 128
                            csz = min(CHUNK_BLOCKS * 128, xg_cols - c0)
                            xg_chunk = xg_pool.tile([128, CHUNK_BLOCKS * 128], F16)
                            nc.sync.dma_start(xg_chunk[:, :csz], d_xg[:, c0:c0 + csz])
                    if qi % 8 == 0:
                        ohb = oh_pool.tile([128, 256], F16)
                        nc.vector.tensor_tensor(
                            out=ohb[:].rearrange("p (a b) -> p a b", b=32),
                            in0=iota_s[:].rearrange("p (a b) -> p a b", b=32),
                            in1=offs_s[:, qi:qi + 8].to_broadcast([128, 8, 32]),
                            op=OP.is_equal)
                    # block-diag lin1: one matmul computes a pair of tiles
                    # (K=66 stacked xg, N=512 block-diagonal weights)
                    mps = []
                    for pr in range(2):
                        mp = ps8.tile([128, 512], F32, tag="mp", bufs=3)
                        blk = (qi >> 1) + pr
                        ccol = 128 * (blk % CHUNK_BLOCKS)
                        nc.tensor.matmul(
                            mp[:], lhsT=xg_chunk[0:66, ccol:ccol + 128],
                            rhs=l1w_s[:], start=True, stop=True)
                        ms = msg_pool.tile([128, 512], F16, tag="ms")
                        if pair_ctr % 12 < 7:
                            nc.scalar.activation(ms[:], mp[:], AF.Relu)
                        else:
                            nc.vector.tensor_scalar_max(ms[:], mp[:], 0.0)
                        pair_ctr += 1
                        mps.append(ms)
                    last = (j == NBLK - 1 and t == T_w - 1)
                    for q in range(4):
                        idx = qi + q
                        g = 4 * j + q
                        for hb in range(2):
                            col = 64 * g + 32 * hb
                            nc.tensor.matmul(
                                agg_b[col // 512][:, col % 512:col % 512 + 32],
                                lhsT=mps[q // 2][:, 256 * (q % 2) + 128 * hb:256 * (q % 2) + 128 * hb + 128],
                                rhs=ohb[:, 32 * (idx % 8):32 * (idx % 8) + 32],
                                start=False, stop=last,
                                skip_group_check=True)

                if j % 2 == 1:
                    # bank j//2 complete: evacuate interleaved halves to aggT
                    src_v = agg_b[j // 2][:].rearrange("p (w t c) -> p w t c",
                                                       t=2, c=32)
                    for hb in range(2):
                        dst = aggT[hb][:, 128 * (j - 1):128 * (j - 1) + 256]
                        dst_v = dst.rearrange("p (w c) -> p w c", c=32)
                        if hb == 0:
                            nc.scalar.activation(dst_v, src_v[:, :, 0, :], AF.Copy)
                        else:
                            nc.vector.tensor_copy(dst_v, src_v[:, :, 1, :])
                if j == 3:
                    emit_gru_chunk(0)
                elif j == 7:
                    emit_gru_chunk(1)

            emit_gru_chunk(2)
            nc.sync.dma_start(
                d_hout[:].rearrange("(k p) n -> p k n", p=128),
                hnew_s[:].rearrange("p (k n) -> p k n", k=2))
            nc.sync.dma_start(d_aout[:], a_s[:])

    nc.compile()
    return nc


def _shared_inputs(inputs):
    lin1_w = inputs["lin1_w"].astype(np.float32)
    lin1_b = inputs["lin1_b"].astype(np.float32)
    w_ih = inputs["w_ih"].astype(np.float32)
    w_hh = inputs["w_hh"].astype(np.float32)
    b_ih = inputs["b_ih"].astype(np.float32)
    b_hh = inputs["b_hh"].astype(np.float32)
    hg_w = inputs["hg_w"].astype(np.float32)
    hg_b = inputs["hg_b"].astype(np.float32)
    ga_w = inputs["ga_w"].astype(np.float32)
    ga_b = inputs["ga_b"].astype(np.float32)

    l1w = np.zeros((66, 512), dtype=np.float32)
    l1w[:32, :256] = lin1_w.T
    l1w[32, :256] = lin1_b
    l1w[33:65, 256:] = lin1_w.T
    l1w[65, 256:] = lin1_b
    wihT = np.ascontiguousarray(w_ih.T)
    whhT = np.ascontiguousarray(w_hh.T)
    f16 = lambda a: np.ascontiguousarray(np.asarray(a, np.float32).astype(NP16))
    f32 = lambda a: np.ascontiguousarray(np.asarray(a, np.float32))
    return {
        "iota": f16(np.tile(np.tile(np.arange(32, dtype=np.float32), 8), (128, 1))),
        "l1w": f16(l1w),
        "wrz": f16(np.concatenate([wihT[:, :512], whhT[:, :512]], axis=0)),
        "win": f16(wihT[:, 512:]),
        "whn": f16(whhT[:, 512:]),
        "hgw": f16(hg_w.T),
        "gawx": f16(ga_w[:, :32].T),
        "gawg": f16(ga_w[:, 32:].T),
        "brz": f32((b_ih[:512] + b_hh[:512]).reshape(512, 1)),
        "bin": f32(b_ih[512:].reshape(256, 1)),
        "bhn": f32(b_hh[512:].reshape(256, 1)),
        "hgb": f32(hg_b.reshape(256, 1)),
        "gab": f32(ga_b.reshape(1, 1)),
    }


def _core_inputs(inputs, T_w, cores):
    x = inputs["x"].astype(np.float32)
    h = inputs["h"].astype(np.float32)
    shared = _shared_inputs(inputs)
    in_maps = []
    for c in range(N_CORES):
        lo = c * NODES_PER_CORE
        hi = min(lo + NODES_PER_CORE, N_NODES)
        ht = np.zeros((256, NODES_PER_CORE), dtype=np.float32)
        ht[:, :hi - lo] = h[lo:hi].T
        xt = np.zeros((32, NODES_PER_CORE), dtype=np.float32)
        xt[:, :hi - lo] = x[lo:hi].T
        m = dict(shared)
        m["xg"] = cores[c]["xg"]
        m["offs"] = cores[c]["offs"]
        m["ht"] = np.ascontiguousarray(ht.astype(NP16))
        m["xt"] = np.ascontiguousarray(xt.astype(NP16))
        in_maps.append(m)
    return in_maps


def kernel(**inputs):
    x = inputs["x"].astype(np.float32)
    T_w, cores = _preprocess(x, inputs["edge_index"])
    if T_w not in _CACHE:
        _CACHE[T_w] = _build_graph(T_w)
    nc = _CACHE[T_w]
    in_maps = _core_inputs(inputs, T_w, cores)
    res = run_bass_kernel_spmd(nc, in_maps, core_ids=list(range(N_CORES)))

    h_new = np.empty((N_NODES, H), dtype=np.float32)
    a = np.empty((N_NODES, 1), dtype=np.float32)
    for c in range(N_CORES):
        lo = c * NODES_PER_CORE
        hi = min(lo + NODES_PER_CORE, N_NODES)
        h_new[lo:hi] = res.results[c]["h_out"][:, :hi - lo].T.astype(np.float32)
        a[lo:hi, 0] = res.results[c]["a_out"][0, :hi - lo]
    a = np.log1p(np.exp(-np.abs(a))) + np.maximum(a, 0.0)  # softplus
    return (a, h_new)


# revision 42
# speedup vs baseline: 1.1635x; 1.0037x over previous
"""Trainium2 Bass kernel for GNN message passing + GRU + MLP head.

Strategy:
  - Sort edges by destination on host; nodes split into 8 ranges of 1280,
    one per NeuronCore -> no collectives.
  - Edges packed into 128-edge tiles grouped by 32-node destination
    windows; windows processed in quads (4 PSUM col-strips) so scatter
    matmuls overlap via tile_position col groups.
  - All matmuls in fp16 (fp32 matmul runs 2-pass LOW_HIGH on TRN2; fp16
    is single-pass with enough mantissa for ~1e-3 end-to-end error).
  - Per tile: lin1 matmul (K=33 incl bias row) -> relu (ACT/DVE split,
    f32 PSUM -> fp16 SBUF) -> one-hot scatter matmul accumulated into
    PSUM-resident agg (has_written cleared by K=1 dummy matmuls).
  - agg transposed on-chip (PE transpose) to feed GRU matmuls; GRU gates
    + MLP head in [H, nodes] layout; outputs un-transposed on host.
"""

import numpy as np
import ml_dtypes

import concourse.bass as bass
import concourse.tile as tile
from concourse import bacc, mybir
from concourse.bass_utils import run_bass_kernel_spmd
from concourse.masks import make_identity

N_NODES = 10000
N_EDGES = 320000
D_IN = 32
H = 256
N_CORES = 8
WIN = 32
NODES_PER_CORE = 1280
N_WIN = NODES_PER_CORE // WIN    # 40 windows/core
NBLK = N_WIN // 4                # 10 window quads
P = 128
F32 = mybir.dt.float32
F16 = mybir.dt.float16
NP16 = np.float16
BF16 = mybir.dt.bfloat16
NPBF = ml_dtypes.bfloat16

_CACHE = {}


def _preprocess(x, edge_index):
    """Sort/partition edges; build per-core packed fp16 inputs."""
    row = np.asarray(edge_index[0], dtype=np.int64)
    col = np.asarray(edge_index[1], dtype=np.int64)
    order = np.argsort(col, kind="stable")
    col_s = col[order]
    row_s = row[order]

    n_win_glob = N_CORES * N_WIN
    bounds = np.searchsorted(col_s, np.arange(0, n_win_glob + 1) * WIN)
    cnt = bounds[1:] - bounds[:-1]
    T_w = max(1, int(np.max((cnt + P - 1) // P)))

    n_tiles = N_WIN * T_w
    e_slot = n_tiles * P
    cores = []
    for c in range(N_CORES):
        src = np.full(e_slot, -1, dtype=np.int64)
        offs = np.full(e_slot, 64.0, dtype=np.float32)
        # tile order: j (quad) -> t (slot) -> q (window in quad)
        idx = 0
        for j in range(NBLK):
            for t in range(T_w):
                for q in range(4):
                    g = 4 * j + q
                    w = c * N_WIN + g
                    lo, hi = bounds[w], bounds[w + 1]
                    s0 = lo + t * P
                    s1 = min(s0 + P, hi)
                    k = s1 - s0
                    if k > 0:
                        b = idx * P
                        src[b:b + k] = row_s[s0:s1]
                        offs[b:b + k] = (col_s[s0:s1] - w * WIN).astype(np.float32)
                    idx += 1
        valid = src >= 0
        xg = np.zeros((e_slot, 33), dtype=np.float32)
        xg[valid, :D_IN] = x[src[valid]]
        xg[:, D_IN] = 1.0  # bias row
        t3 = xg.reshape(n_tiles, P, 33).transpose(0, 2, 1)  # [T, 33, 128]
        pack = np.zeros((n_tiles // 2, 128, 128), dtype=np.float32)
        pack[:, 0:33, :] = t3[0::2]
        pack[:, 33:66, :] = t3[1::2]
        xg_pack = pack.transpose(1, 0, 2).reshape(128, (n_tiles // 2) * 128)
        offs_arr = offs.reshape(n_tiles, P).T  # [128, T]
        cores.append({
            "xg": np.ascontiguousarray(xg_pack.astype(NP16)),
            "offs": np.ascontiguousarray(offs_arr.astype(NP16)),
        })
    return T_w, cores


def _build_graph(T_w):
    n_tiles = N_WIN * T_w
    xg_cols = (n_tiles // 2) * 128
    NPAD = NODES_PER_CORE

    nc = bacc.Bacc()
    d_xg = nc.declare_dram_parameter("xg", [128, xg_cols], F16, isOutput=False)
    d_offs = nc.declare_dram_parameter("offs", [128, n_tiles], F16, isOutput=False)
    d_iota = nc.declare_dram_parameter("iota", [128, 256], F16, isOutput=False)
    d_l1w = nc.declare_dram_parameter("l1w", [66, 512], F16, isOutput=False)
    d_ht = nc.declare_dram_parameter("ht", [256, NPAD], F16, isOutput=False)
    d_xt = nc.declare_dram_parameter("xt", [32, NPAD], F16, isOutput=False)
    d_wrz = nc.declare_dram_parameter("wrz", [512, 512], F16, isOutput=False)
    d_win = nc.declare_dram_parameter("win", [256, 256], F16, isOutput=False)
    d_whn = nc.declare_dram_parameter("whn", [256, 256], F16, isOutput=False)
    d_hgw = nc.declare_dram_parameter("hgw", [256, 256], F16, isOutput=False)
    d_gawx = nc.declare_dram_parameter("gawx", [32, 1], F16, isOutput=False)
    d_gawg = nc.declare_dram_parameter("gawg", [256, 1], F16, isOutput=False)
    d_brz = nc.declare_dram_parameter("brz", [512, 1], F32, isOutput=False)
    d_bin = nc.declare_dram_parameter("bin", [256, 1], F32, isOutput=False)
    d_bhn = nc.declare_dram_parameter("bhn", [256, 1], F32, isOutput=False)
    d_hgb = nc.declare_dram_parameter("hgb", [256, 1], F32, isOutput=False)
    d_gab = nc.declare_dram_parameter("gab", [1, 1], F32, isOutput=False)
    d_hout = nc.declare_dram_parameter("h_out", [256, NPAD], F16, isOutput=True)
    d_aout = nc.declare_dram_parameter("a_out", [1, NPAD], F32, isOutput=True)

    AF = mybir.ActivationFunctionType
    OP = mybir.AluOpType

    with tile.TileContext(nc) as tc:
        with (
            tc.tile_pool(name="const", bufs=1) as cpool,
            tc.tile_pool(name="ps8", bufs=1, space="PSUM") as ps8,
            tc.tile_pool(name="xgc", bufs=4) as xg_pool,
            tc.tile_pool(name="ohb", bufs=6) as oh_pool,
            tc.tile_pool(name="msgs", bufs=8) as msg_pool,
            tc.tile_pool(name="gsb", bufs=14) as gsb,
        ):
            # ---- first xg tiles first: minimize PE start latency ----
            xg_first = xg_pool.tile([128, 8 * 128], F16, tag="xgf", bufs=1)
            for pc in range(4):
                nc.sync.dma_start(xg_first[:, 256 * pc:256 * pc + 256],
                                  d_xg[:, 256 * pc:256 * pc + 256])
            # ---- constants / inputs (sync queue: edge data first) ----
            iota_s = cpool.tile([128, 256], F16)
            nc.sync.dma_start(iota_s[:], d_iota[:])
            l1w_s = cpool.tile([66, 512], F16)  # block-diag for tile pairs
            nc.sync.dma_start(l1w_s[:], d_l1w[:])
            offs_s = cpool.tile([128, n_tiles], F16)
            nc.sync.dma_start(offs_s[:], d_offs[:])
            zc = cpool.tile([1, 128], F16)
            nc.vector.memset(zc[:], 0.0)
            zr = cpool.tile([1, 512], F16)
            nc.vector.memset(zr[:], 0.0)

            # ---- GRU weights / node inputs (gpsimd queue) ----
            ht_s = cpool.tile([128, 2 * NPAD], F16)
            nc.gpsimd.dma_start(
                ht_s[:].rearrange("p (k n) -> p k n", k=2),
                d_ht[:].rearrange("(k p) n -> p k n", p=128))
            xt_s = cpool.tile([32, NPAD], F16)
            nc.gpsimd.dma_start(xt_s[:], d_xt[:])
            wrz_s = cpool.tile([128, 2048], F16)
            nc.gpsimd.dma_start(
                wrz_s[:].rearrange("p (k m) -> p k m", k=4),
                d_wrz[:].rearrange("(k p) m -> p k m", p=128))
            win_s = cpool.tile([128, 512], F16)
            nc.gpsimd.dma_start(
                win_s[:].rearrange("p (k m) -> p k m", k=2),
                d_win[:].rearrange("(k p) m -> p k m", p=128))
            whn_s = cpool.tile([128, 512], F16)
            nc.gpsimd.dma_start(
                whn_s[:].rearrange("p (k m) -> p k m", k=2),
                d_whn[:].rearrange("(k p) m -> p k m", p=128))
            hgw_s = cpool.tile([128, 512], F16)
            nc.gpsimd.dma_start(
                hgw_s[:].rearrange("p (k m) -> p k m", k=2),
                d_hgw[:].rearrange("(k p) m -> p k m", p=128))
            gawx_s = cpool.tile([32, 1], F16)
            nc.gpsimd.dma_start(gawx_s[:], d_gawx[:])
            gawg_s = cpool.tile([128, 2], F16)
            nc.gpsimd.dma_start(
                gawg_s[:].rearrange("p (k o) -> p k o", k=2),
                d_gawg[:].rearrange("(k p) o -> p k o", p=128))
            brz_s = cpool.tile([128, 4], F32)
            nc.gpsimd.dma_start(
                brz_s[:].rearrange("p (k o) -> p k o", k=4),
                d_brz[:].rearrange("(k p) o -> p k o", p=128))
            bin_s = cpool.tile([128, 2], F32)
            nc.gpsimd.dma_start(
                bin_s[:].rearrange("p (k o) -> p k o", k=2),
                d_bin[:].rearrange("(k p) o -> p k o", p=128))
            bhn_s = cpool.tile([128, 2], F32)
            nc.gpsimd.dma_start(
                bhn_s[:].rearrange("p (k o) -> p k o", k=2),
                d_bhn[:].rearrange("(k p) o -> p k o", p=128))
            hgb_s = cpool.tile([128, 2], F32)
            nc.gpsimd.dma_start(
                hgb_s[:].rearrange("p (k o) -> p k o", k=2),
                d_hgb[:].rearrange("(k p) o -> p k o", p=128))
            gab_s = cpool.tile([1, 1], F32)
            nc.gpsimd.dma_start(gab_s[:], d_gab[:])

            # ---- persistent SBUF state ----
            aggT = [cpool.tile([128, NPAD], F16, name=f"aggT{i}") for i in range(2)]
            hts = [ht_s[:, :NPAD], ht_s[:, NPAD:]]
            hnew_s = cpool.tile([128, 2 * NPAD], F16)
            hnews = [hnew_s[:, :NPAD], hnew_s[:, NPAD:]]
            a_s = cpool.tile([1, NPAD], F32)

            # ---- agg PSUM banks (slots recycle into GRU psum) ----
            agg_b = [ps8.tile([128, 512], F32, tag="agg", bufs=5, name=f"agg_b{k}")
                     for k in range(5)]
            for k in range(5):  # clear has_written bits
                nc.tensor.matmul(agg_b[k][:], lhsT=zc[:], rhs=zr[:],
                                 start=True, stop=False, skip_group_check=True)

            kstack = [aggT[0][:], aggT[1][:], hts[0], hts[1]]
            NCHUNK = [(0, 512), (512, 1024), (1024, NPAD)]

            def emit_gru_chunk(ci):
                n0, n1 = NCHUNK[ci]
                ncn = n1 - n0
                rz_ps = []
                for m in range(4):
                    ps = ps8.tile([128, 512], F32, tag="agg", bufs=5, name=f"rz{ci}{m}")
                    for k in (2, 3, 0, 1):  # h-side first
                        nc.tensor.matmul(
                            ps[:, :ncn], lhsT=wrz_s[:, 512 * k + 128 * m:512 * k + 128 * m + 128],
                            rhs=kstack[k][:, n0:n1], start=(k == 2), stop=(k == 1))
                    rz_ps.append(ps)
                hn_ps = []
                for m in range(2):
                    ps = ps8.tile([128, 512], F32, tag="agg", bufs=5, name=f"hn{ci}{m}")
                    for k in range(2):
                        nc.tensor.matmul(
                            ps[:, :ncn], lhsT=whn_s[:, 256 * k + 128 * m:256 * k + 128 * m + 128],
                            rhs=hts[k][:, n0:n1], start=(k == 0), stop=(k == 1))
                    hn_ps.append(ps)
                in_ps = []
                for m in range(2):
                    ps = ps8.tile([128, 512], F32, tag="agg", bufs=5, name=f"in{ci}{m}")
                    for k in range(2):
                        nc.tensor.matmul(
                            ps[:, :ncn], lhsT=win_s[:, 256 * k + 128 * m:256 * k + 128 * m + 128],
                            rhs=aggT[k][:, n0:n1], start=(k == 0), stop=(k == 1))
                    in_ps.append(ps)

                n_sb = []
                z_sb = []
                for m in range(2):
                    r_m = gsb.tile([128, 512], F16, tag="g")
                    nc.scalar.activation(r_m[:, :ncn], rz_ps[m][:, :ncn],
                                         AF.Sigmoid, bias=brz_s[:, m:m + 1])
                    z_m = gsb.tile([128, 512], F16, tag="g")
                    nc.scalar.activation(z_m[:, :ncn], rz_ps[2 + m][:, :ncn],
                                         AF.Sigmoid, bias=brz_s[:, 2 + m:3 + m])
                    z_sb.append(z_m)
                    t1 = gsb.tile([128, 512], F16, tag="g")
                    nc.vector.scalar_tensor_tensor(
                        out=t1[:, :ncn], in0=hn_ps[m][:, :ncn],
                        scalar=bhn_s[:, m:m + 1], in1=r_m[:, :ncn],
                        op0=OP.add, op1=OP.mult)
                    t2 = gsb.tile([128, 512], F16, tag="g")
                    nc.vector.tensor_tensor(out=t2[:, :ncn], in0=t1[:, :ncn],
                                            in1=in_ps[m][:, :ncn], op=OP.add)
                    n_m = gsb.tile([128, 512], F16, tag="g")
                    nc.scalar.activation(n_m[:, :ncn], t2[:, :ncn],
                                         AF.Tanh, bias=bin_s[:, m:m + 1])
                    n_sb.append(n_m)
                for m in range(2):
                    d_m = gsb.tile([128, 512], F16, tag="g")
                    nc.vector.tensor_tensor(out=d_m[:, :ncn], in0=hts[m][:, n0:n1],
                                            in1=n_sb[m][:, :ncn], op=OP.subtract)
                    e_m = gsb.tile([128, 512], F16, tag="g")
                    nc.vector.tensor_tensor(out=e_m[:, :ncn], in0=z_sb[m][:, :ncn],
                                            in1=d_m[:, :ncn], op=OP.mult)
                    nc.vector.tensor_tensor(out=hnews[m][:, n0:n1], in0=n_sb[m][:, :ncn],
                                            in1=e_m[:, :ncn], op=OP.add)
                a_ps = ps8.tile([1, 512], F32, tag="agg", bufs=5, name=f"aps{ci}")
                nc.tensor.matmul(a_ps[:, :ncn], lhsT=gawx_s[:],
                                 rhs=xt_s[:, n0:n1], start=True, stop=False,
                                 skip_group_check=True)
                for m in range(2):
                    g_ps = ps8.tile([128, 512], F32, tag="agg", bufs=5, name=f"gps{ci}{m}")
                    for k in range(2):
                        nc.tensor.matmul(
                            g_ps[:, :ncn], lhsT=hgw_s[:, 256 * k + 128 * m:256 * k + 128 * m + 128],
                            rhs=hnews[k][:, n0:n1], start=(k == 0), stop=(k == 1))
                    g_m = gsb.tile([128, 512], F16, tag="g")
                    nc.vector.tensor_scalar(
                        out=g_m[:, :ncn], in0=g_ps[:, :ncn],
                        scalar1=hgb_s[:, m:m + 1], scalar2=0.0,
                        op0=OP.add, op1=OP.max)
                    nc.tensor.matmul(a_ps[:, :ncn], lhsT=gawg_s[:, m:m + 1],
                                     rhs=g_m[:, :ncn], start=False,
                                     stop=(m == 1), skip_group_check=True)
                nc.scalar.activation(a_s[:, n0:n1], a_ps[:, :ncn],
                                     AF.Identity, bias=gab_s[:])

            # ---- edge phase (GRU chunks interleaved as agg banks free) ----
            CHUNK_BLOCKS = 8  # 16 tiles per DMA
            xg_chunk = None
            ohb = None
            pair_ctr = 0
            for j in range(NBLK):
                for t in range(T_w):
                    qi = (j * T_w + t) * 4
                    if qi % 16 == 0:
                        if qi == 0:
                            xg_chunk = xg_first
                        else:
                            c0 = (qi // 2) * 128
                            csz = min(CHUNK_BLOCKS * 128, xg_cols - c0)
                            xg_chunk = xg_pool.tile([128, CHUNK_BLOCKS * 128], F16)
                            nc.sync.dma_start(xg_chunk[:, :csz], d_xg[:, c0:c0 + csz])
                    if qi % 8 == 0:
                        ohb = oh_pool.tile([128, 256], F16)
                        nc.vector.tensor_tensor(
                            out=ohb[:].rearrange("p (a b) -> p a b", b=32),
                            in0=iota_s[:].rearrange("p (a b) -> p a b", b=32),
                            in1=offs_s[:, qi:qi + 8].to_broadcast([128, 8, 32]),
                            op=OP.is_equal)
                    # block-diag lin1: one matmul computes a pair of tiles
                    # (K=66 stacked xg, N=512 block-diagonal weights)
                    mps = []
                    for pr in range(2):
                        mp = ps8.tile([128, 512], F32, tag="mp", bufs=3)
                        blk = (qi >> 1) + pr
                        ccol = 128 * (blk % CHUNK_BLOCKS)
                        nc.tensor.matmul(
                            mp[:], lhsT=xg_chunk[0:66, ccol:ccol + 128],
                            rhs=l1w_s[:], start=True, stop=True)
                        ms = msg_pool.tile([128, 512], F16, tag="ms")
                        if pair_ctr % 12 < 7:
                            nc.scalar.activation(ms[:], mp[:], AF.Relu)
                        else:
                            nc.vector.tensor_scalar_max(ms[:], mp[:], 0.0)
                        pair_ctr += 1
                        mps.append(ms)
                    last = (j == NBLK - 1 and t == T_w - 1)
                    for q in range(4):
                        idx = qi + q
                        g = 4 * j + q
                        for hb in range(2):
                            col = 64 * g + 32 * hb
                            nc.tensor.matmul(
                                agg_b[col // 512][:, col % 512:col % 512 + 32],
                                lhsT=mps[q // 2][:, 256 * (q % 2) + 128 * hb:256 * (q % 2) + 128 * hb + 128],
                                rhs=ohb[:, 32 * (idx % 8):32 * (idx % 8) + 32],
                                start=False, stop=last,
                                skip_group_check=True)

                if j % 2 == 1:
                    # bank j//2 complete: evacuate interleaved halves to aggT
                    src_v = agg_b[j // 2][:].rearrange("p (w t c) -> p w t c",
                                                       t=2, c=32)
                    for hb in range(2):
                        dst = aggT[hb][:, 128 * (j - 1):128 * (j - 1) + 256]
                        dst_v = dst.rearrange("p (w c) -> p w c", c=32)
                        if hb == 0:
                            nc.scalar.activation(dst_v, src_v[:, :, 0, :], AF.Copy)
                        else:
                            nc.vector.tensor_copy(dst_v, src_v[:, :, 1, :])
                if j == 3:
                    emit_gru_chunk(0)
                elif j == 7:
                    emit_gru_chunk(1)

            emit_gru_chunk(2)
            nc.sync.dma_start(
                d_hout[:].rearrange("(k p) n -> p k n", p=128),
                hnew_s[:].rearrange("p (k n) -> p k n", k=2))
            nc.sync.dma_start(d_aout[:], a_s[:])

    nc.compile()
    return nc


def _shared_inputs(inputs):
    lin1_w = inputs["lin1_w"].astype(np.float32)
    lin1_b = inputs["lin1_b"].astype(np.float32)
    w_ih = inputs["w_ih"].astype(np.float32)
    w_hh = inputs["w_hh"].astype(np.float32)
    b_ih = inputs["b_ih"].astype(np.float32)
    b_hh = inputs["b_hh"].astype(np.float32)
    hg_w = inputs["hg_w"].astype(np.float32)
    hg_b = inputs["hg_b"].astype(np.float32)
    ga_w = inputs["ga_w"].astype(np.float32)
    ga_b = inputs["ga_b"].astype(np.float32)

    l1w = np.zeros((66, 512), dtype=np.float32)
    l1w[:32, :256] = lin1_w.T
    l1w[32, :256] = lin1_b
    l1w[33:65, 256:] = lin1_w.T
    l1w[65, 256:] = lin1_b
    wihT = np.ascontiguousarray(w_ih.T)
    whhT = np.ascontiguousarray(w_hh.T)
    f16 = lambda a: np.ascontiguousarray(np.asarray(a, np.float32).astype(NP16))
    f32 = lambda a: np.ascontiguousarray(np.asarray(a, np.float32))
    return {
        "iota": f16(np.tile(np.tile(np.arange(32, dtype=np.float32), 8), (128, 1))),
        "l1w": f16(l1w),
        "wrz": f16(np.concatenate([wihT[:, :512], whhT[:, :512]], axis=0)),
        "win": f16(wihT[:, 512:]),
        "whn": f16(whhT[:, 512:]),
        "hgw": f16(hg_w.T),
        "gawx": f16(ga_w[:, :32].T),
        "gawg": f16(ga_w[:, 32:].T),
        "brz": f32((b_ih[:512] + b_hh[:512]).reshape(512, 1)),
        "bin": f32(b_ih[512:].reshape(256, 1)),
        "bhn": f32(b_hh[512:].reshape(256, 1)),
        "hgb": f32(hg_b.reshape(256, 1)),
        "gab": f32(ga_b.reshape(1, 1)),
    }


def _core_inputs(inputs, T_w, cores):
    x = inputs["x"].astype(np.float32)
    h = inputs["h"].astype(np.float32)
    shared = _shared_inputs(inputs)
    in_maps = []
    for c in range(N_CORES):
        lo = c * NODES_PER_CORE
        hi = min(lo + NODES_PER_CORE, N_NODES)
        ht = np.zeros((256, NODES_PER_CORE), dtype=np.float32)
        ht[:, :hi - lo] = h[lo:hi].T
        xt = np.zeros((32, NODES_PER_CORE), dtype=np.float32)
        xt[:, :hi - lo] = x[lo:hi].T
        m = dict(shared)
        m["xg"] = cores[c]["xg"]
        m["offs"] = cores[c]["offs"]
        m["ht"] = np.ascontiguousarray(ht.astype(NP16))
        m["xt"] = np.ascontiguousarray(xt.astype(NP16))
        in_maps.append(m)
    return in_maps


def kernel(**inputs):
    x = inputs["x"].astype(np.float32)
    T_w, cores = _preprocess(x, inputs["edge_index"])
    if T_w not in _CACHE:
        _CACHE[T_w] = _build_graph(T_w)
    nc = _CACHE[T_w]
    in_maps = _core_inputs(inputs, T_w, cores)
    res = run_bass_kernel_spmd(nc, in_maps, core_ids=list(range(N_CORES)))

    h_new = np.empty((N_NODES, H), dtype=np.float32)
    a = np.empty((N_NODES, 1), dtype=np.float32)
    for c in range(N_CORES):
        lo = c * NODES_PER_CORE
        hi = min(lo + NODES_PER_CORE, N_NODES)
        h_new[lo:hi] = res.results[c]["h_out"][:, :hi - lo].T.astype(np.float32)
        a[lo:hi, 0] = res.results[c]["a_out"][0, :hi - lo]
    a = np.log1p(np.exp(-np.abs(a))) + np.maximum(a, 0.0)  # softplus
    return (a, h_new)
